# revision 21
# baseline (speedup 1.0000x reference)
"""DepthTransformer Trainium2 kernel.

Strategy: data-parallel over the batch dim b=6 across cores (cores 6,7
process duplicate samples; host discards). Per core, one full sample:

  xp   = silu(gn_in(W_in @ x + b_in))                      [320, 1024]
  cdup = relu(gn_ctx(W_ctx @ ctx)) duplicated on 128 partitions (j2)
  Attention (software-pipelined over 8 pixel tiles; the ctxa/wov stage
  lags the sim/softmax stage by one tile so the DVE never waits on the
  PE->scalar softmax chain):
    qtjdup[j2, h2, px]: j-major folded q (wk^T wq xp), head pairs on
      partition halves.
    sim: prod_pair = cdup * qtjdup (DVE 2x bf16); the j-reduction runs
      on the PE via pair-mask matmuls at 4 concurrent tile_position
      col-groups; exp() fused into the PSUM evac (no max subtraction --
      |sim| << 1 for this net); PE transposes (psel) return the scores
      to pixel-major [px, d, n]; softmax denominator on DVE.
    ctxa: pixel-major prod2 + in-place tree reduction over d (DVE),
      using tjd (PE-transposed cdup slices).
    wov projection via PE transposes of ctxa + matmuls.
  y1 = relu(gn1(out1)); y2 = relu(gn2(conv3x3(y1, w1)))
  y  = conv3x3(y2, w2) + x   (residual via identity matmul in PSUM)

GroupNorm stats: scalar-engine evac accum_out for sums + scalar Square
passes for sumsq (ctx) / DVE square (xp, out1, y2). Group aggregation
via small matmuls (exact f32 weights).
"""

import os
import numpy as np
import ml_dtypes

import concourse.bass as bass
import concourse.bacc as bacc
import concourse.tile as tile
from concourse import mybir
from concourse.bass_utils import run_bass_kernel_spmd

F32 = mybir.dt.float32
BF16 = mybir.dt.bfloat16
AF = mybir.ActivationFunctionType
ALU = mybir.AluOpType
AX = mybir.AxisListType

HN, HD, CD, D = 8, 40, 64, 32
CH = HN * HD          # 320
NPIX = 1024           # 32*32
NT = 3                # channel tiles of 128 for 320 (128,128,64 padded to 384)
EPS = 1e-5
DEBUG = bool(int(os.environ.get("DT_DEBUG", "0")))


def _bcast(ap, axis, n):
    """Insert a step-0 broadcast dim of size n at free-dim position `axis`."""
    return ap.unsqueeze(axis).broadcast_to(
        tuple(ap.shape[:axis]) + (n,) + tuple(ap.shape[axis:]))


def build_program(with_pos):
    nc = bacc.Bacc("TRN2", target_bir_lowering=False, debug=False)

    def inp(name, shape, dt=F32):
        return nc.dram_tensor(name, shape, dt, kind="ExternalInput").ap()

    xbf_d = inp("x_bf", [384, NPIX], BF16)
    ctx_d = inp("ctxin", [CD, D * NPIX], BF16)
    w_in_t = inp("w_in_t", [NT, NT, 128, 128], BF16)
    b_in = inp("b_in", [384, 1])
    gin_g = inp("gin_g", [384, 1])
    gin_b = inp("gin_b", [384, 1])
    wctxdup_d = inp("wctxdup", [128, 128], BF16)
    gctx_g = inp("gctx_g", [128, 1])
    gctx_b = inp("gctx_b", [128, 1])
    wqktj_d = inp("wqktj", [HN, NT, 128, CD], BF16)
    if with_pos:
        pos_dj = inp("pos_dup", [128, D])   # posT dup'd: [j2, d] f32
    wovt_d = inp("wovt", [4, NT, 128, 128], BF16)
    g1_g = inp("g1_g", [384, 1])
    g1_b = inp("g1_b", [384, 1])
    g2_g = inp("g2_g", [384, 1])
    g2_b = inp("g2_b", [384, 1])
    c1_d = inp("conv1_t", [NT, NT, 128, 9, 128], BF16)
    c2_d = inp("conv2_t", [NT, NT, 128, 9, 128], BF16)
    gsel8_d = inp("gsel8", [NT, 128, 8])
    gselT_d = inp("gselT", [NT, 8, 128])
    g2c_d = inp("g2ctx", [128, 128])
    eye_d = inp("eye", [128, 128], BF16)
    mask32_d = inp("mask32", [128, 32], BF16)
    psel_d = inp("psel", [98, 8], BF16)

    y_d = nc.dram_tensor("y", [CH, NPIX], F32, kind="ExternalOutput").ap()

    dbg = {}
    if DEBUG:
        for nm, shape, dt in [
            ("xp_dbg", [CH, NPIX], BF16),
            ("ctxpf_dbg", [CD, D * NPIX], BF16),
            ("qtj_dbg", [128, 4 * NPIX], BF16),
            ("a_dbg", [NPIX, 256], BF16),
            ("ctxa_dbg", [NPIX, 512], BF16),
            ("out1_dbg", [CH, NPIX], BF16),
        ]:
            dbg[nm] = nc.dram_tensor(nm, shape, dt, kind="ExternalOutput").ap()

    with tile.TileContext(nc) as tc:
        from contextlib import ExitStack

        es = ExitStack()
        persist = es.enter_context(tc.tile_pool(name="persist", bufs=1))
        wpool = es.enter_context(tc.tile_pool(name="wpool", bufs=1))
        wstream = es.enter_context(tc.tile_pool(name="wstream", bufs=1))
        stage = es.enter_context(tc.tile_pool(name="stage", bufs=3))
        small = es.enter_context(tc.tile_pool(name="small", bufs=2))
        attn = es.enter_context(tc.tile_pool(name="attn", bufs=2))
        psum = es.enter_context(tc.tile_pool(name="psum", bufs=2, space="PSUM"))
        spsum = es.enter_context(tc.tile_pool(name="spsum", bufs=1, space="PSUM"))
        tpsum = es.enter_context(tc.tile_pool(name="tpsum", bufs=1, space="PSUM"))
        cpsum = es.enter_context(tc.tile_pool(name="cpsum", bufs=1, space="PSUM"))
        mpsum = es.enter_context(tc.tile_pool(name="mpsum", bufs=2, space="PSUM"))

        # ---------------- load x + proj_in weights first ----------------
        _wn = [0]
        def load_w(pool, src, shape, dt=F32, tag=None, eng=None):
            _wn[0] += 1
            t = pool.tile(shape, dt, tag=tag or "", name=f"w{_wn[0]}")
            (eng or nc.sync).dma_start(out=t[:], in_=src)
            return t

        xbf_sb = [persist.tile([128, NPIX], BF16, name=f"xb_{t}") for t in range(NT)]
        for t in range(NT):
            nc.sync.dma_start(out=xbf_sb[t][:], in_=xbf_d[t * 128:(t + 1) * 128, :])
        win_sb = [[load_w(wpool, w_in_t[k, m], [128, 128], BF16,
                          tag=f"wsh{k * NT + m}") for m in range(NT)]
                  for k in range(NT)]
        bin_sb = [load_w(wpool, b_in[m * 128:(m + 1) * 128], [128, 1]) for m in range(NT)]

        wctxdup_sb = load_w(wpool, wctxdup_d, [128, 128], BF16)
        eye_sb = load_w(wpool, eye_d, [128, 128], BF16)

        def load_vec(src):
            return [load_w(wpool, src[m * 128:(m + 1) * 128], [128, 1]) for m in range(NT)]

        gin_g_sb, gin_b_sb = load_vec(gin_g), load_vec(gin_b)
        gctx_g_sb = load_w(wpool, gctx_g, [128, 1])
        gctx_b_sb = load_w(wpool, gctx_b, [128, 1])

        eps_sb = wpool.tile([128, 1], F32)
        nc.vector.memset(eps_sb[:], EPS)

        if with_pos:
            posdup_sb = load_w(wpool, pos_dj, [128, D])

        _DEF = [None]

        def load_deferred():
            # weights not needed until phase 3+: issue their DMAs after the
            # ctx stream so input chunks aren't queued behind them
            E = nc.scalar
            w = {}
            w["wqktj"] = [[load_w(wpool, wqktj_d[n, k], [128, CD], BF16, eng=E)
                           for k in range(NT)] for n in range(HN)]
            w["wovt"] = [[load_w(wpool, wovt_d[k, m], [128, 128], BF16, eng=E,
                                 tag=f"wsh{k * NT + m}" if k * NT + m < 9 else None)
                          for m in range(NT)] for k in range(4)]
            w["gsel8"] = [load_w(wpool, gsel8_d[k], [128, 8], eng=E) for k in range(NT)]
            w["gselT"] = [load_w(wpool, gselT_d[m], [8, 128], eng=E) for m in range(NT)]
            w["g2c"] = load_w(wpool, g2c_d, [128, 128], eng=E)
            w["mask32"] = load_w(wpool, mask32_d, [128, 32], BF16, eng=E)
            w["psel"] = load_w(wpool, psel_d, [98, 8], BF16, eng=E)
            w["g1"] = ([load_w(wpool, g1_g[m * 128:(m + 1) * 128], [128, 1], eng=E)
                        for m in range(NT)],
                       [load_w(wpool, g1_b[m * 128:(m + 1) * 128], [128, 1], eng=E)
                        for m in range(NT)])
            w["g2"] = ([load_w(wpool, g2_g[m * 128:(m + 1) * 128], [128, 1], eng=E)
                        for m in range(NT)],
                       [load_w(wpool, g2_b[m * 128:(m + 1) * 128], [128, 1], eng=E)
                        for m in range(NT)])
            return w

        # =========== GN finalize (320-channel, 8 groups of 40) ===========
        def gn_affine_320(m2, gamma_sb, beta_sb, name):
            gp8 = spsum.tile([128, 2], F32, tag="gnps")
            for k in range(NT):
                nc.tensor.matmul(gp8[:8, :], gsel8_sb[k][:], m2[k][:],
                                 start=(k == 0), stop=(k == NT - 1))
            g8 = small.tile([8, 2], F32, tag=f"{name}_g8")
            nc.vector.tensor_copy(g8[:], gp8[:8, :])
            s_t = []
            for m in range(NT):
                gp = spsum.tile([128, 2], F32, tag="gnps")
                nc.tensor.matmul(gp[:], gselT_sb[m][:], g8[:], start=True, stop=True)
                gs = small.tile([128, 2], F32, tag=f"{name}_gs{m}")
                nc.vector.tensor_copy(gs[:], gp[:])
                s = small.tile([128, 1], F32, tag=f"{name}_s{m}")
                tt = small.tile([128, 1], F32, tag=f"{name}_t{m}")
                vt = small.tile([128, 1], F32, tag=f"{name}_v")
                nc.vector.tensor_mul(vt[:], gs[:, 0:1], gs[:, 0:1])
                nc.vector.tensor_sub(vt[:], gs[:, 1:2], vt[:])
                nc.scalar.activation(out=vt[:], in_=vt[:], func=AF.Sqrt, bias=eps_sb[:, 0:1])
                nc.vector.reciprocal(out=vt[:], in_=vt[:])
                nc.vector.tensor_mul(s[:], gamma_sb[m][:], vt[:])
                nc.vector.tensor_mul(tt[:], gs[:, 0:1], s[:])
                nc.vector.tensor_sub(tt[:], beta_sb[m][:], tt[:])
                s_t.append((s, tt))
            return s_t

        # build (sum, sumsq) m2 tiles from evac accum cols + one square pass
        def make_m2(src_bf, acc, name, sq_acc=None):
            m2 = []
            for m in range(NT):
                t = small.tile([128, 2], F32, tag=f"{name}_m2{m}", name=f"{name}_m2{m}")
                nc.vector.tensor_reduce(out=t[:, 0:1], in_=acc[m][:], axis=AX.X, op=ALU.add)
                if sq_acc is not None:
                    nc.vector.tensor_reduce(out=t[:, 1:2], in_=sq_acc[m][:], axis=AX.X,
                                            op=ALU.add)
                else:
                    sq = stage.tile([128, NPIX], BF16, tag="sqjunk", bufs=1)
                    nc.vector.scalar_tensor_tensor(
                        out=sq[:], in0=src_bf[m][:], scalar=1.0, in1=src_bf[m][:],
                        op0=ALU.mult, op1=ALU.mult, accum_out=t[:, 1:2])
                m2.append(t)
            return m2

        _DEF[0] = load_deferred()

        # ---------------- phase 1: proj_in ----------------
        xpr = [persist.tile([128, NPIX], BF16, tag=f"o1_{t}", name=f"xpr_{t}") for t in range(NT)]
        xac = [small.tile([128, 2], F32, tag=f"xac{m}", name=f"xac{m}") for m in range(NT)]
        for m in range(NT):
            for n in range(2):
                ps = psum.tile([128, 512], F32, tag="mm")
                for k in range(NT):
                    nc.tensor.matmul(ps[:], win_sb[k][m][:], xbf_sb[k][:, n * 512:(n + 1) * 512],
                                     start=(k == 0), stop=(k == NT - 1))
                nc.scalar.activation(out=xpr[m][:, n * 512:(n + 1) * 512], in_=ps[:],
                                     func=AF.Identity, bias=bin_sb[m][:, 0:1],
                                     accum_out=xac[m][:, n:n + 1])

        # ---------------- phase 2: cdup (dup ctx proj, raw) + DVE stats ----------------
        cdup = persist.tile([128, D, NPIX], BF16, name="cdup")
        cdup_fl = cdup[:].rearrange("j d p -> j (d p)")
        NCH = 64
        cst = persist.tile([128, NCH, 6], F32, name="cst")
        for cc in range(NCH // 2):
            cin = stage.tile([128, 512], BF16, tag="ctxin", bufs=3)
            nc.sync.dma_start(out=cin[0:CD, :], in_=ctx_d[:, (2 * cc) * 512:(2 * cc + 1) * 512])
            nc.sync.dma_start(out=cin[CD:128, :], in_=ctx_d[:, (2 * cc + 1) * 512:(2 * cc + 2) * 512])
            for half in range(2):
                c = 2 * cc + half
                ps = psum.tile([128, 512], F32, tag="mm")
                nc.tensor.matmul(ps[:], wctxdup_sb[half * CD:(half + 1) * CD, :],
                                 cin[half * CD:(half + 1) * CD, :], start=True, stop=True)
                nc.scalar.activation(out=cdup_fl[:, c * 512:(c + 1) * 512], in_=ps[:],
                                     func=AF.Copy)
                nc.vector.bn_stats(out=cst[:, c, :], in_=cdup_fl[:, c * 512:(c + 1) * 512])
        wdef = _DEF[0]
        wqktj_sb, wovt_sb = wdef["wqktj"], wdef["wovt"]
        gsel8_sb, gselT_sb, g2c_sb = wdef["gsel8"], wdef["gselT"], wdef["g2c"]
        mask32_sb, psel_sb = wdef["mask32"], wdef["psel"]
        g1_g_sb, g1_b_sb = wdef["g1"]
        g2_g_sb, g2_b_sb = wdef["g2"]
        cmv = small.tile([128, 2], F32)
        nc.vector.bn_aggr(out=cmv[:], in_=cst[:])
        cm2 = small.tile([128, 2], F32)
        nc.vector.tensor_copy(cm2[:, 0:1], cmv[:, 0:1])
        nc.vector.tensor_mul(cm2[:, 1:2], cmv[:, 0:1], cmv[:, 0:1])
        nc.vector.tensor_add(cm2[:, 1:2], cm2[:, 1:2], cmv[:, 1:2])
        # group aggregate: g2c entries are 1/8 -> cgs = (mean, E2)
        cgp = spsum.tile([128, 2], F32, tag="gnps")
        nc.tensor.matmul(cgp[:], g2c_sb[:], cm2[:], start=True, stop=True)
        cgs = small.tile([128, 2], F32)
        nc.vector.tensor_copy(cgs[:], cgp[:])
        cs = small.tile([128, 1], F32)
        ct = small.tile([128, 1], F32)
        cv = small.tile([128, 1], F32)
        nc.vector.tensor_mul(cv[:], cgs[:, 0:1], cgs[:, 0:1])
        nc.vector.tensor_sub(cv[:], cgs[:, 1:2], cv[:])
        nc.scalar.activation(out=cv[:], in_=cv[:], func=AF.Sqrt, bias=eps_sb[:, 0:1])
        nc.vector.reciprocal(out=cv[:], in_=cv[:])
        nc.vector.tensor_mul(cs[:], gctx_g_sb[:], cv[:])
        nc.vector.tensor_mul(ct[:], cgs[:, 0:1], cs[:])
        nc.vector.tensor_sub(ct[:], gctx_b_sb[:], ct[:])
        if with_pos:
            # ctpos[j2, d] = ct[j2] + posT[j, d] (bias per (partition, d-slice))
            ctpos = small.tile([128, D], F32, name="ctpos")
            nc.vector.tensor_scalar_add(out=ctpos[:], in0=posdup_sb[:],
                                        scalar1=ct[:, 0:1])

        # ---------------- phase 1b: gn_in finalize + silu (in place) ----------------
        st_in = gn_affine_320(make_m2(xpr, xac, "gin"), gin_g_sb, gin_b_sb, "gin")
        for m in range(NT):
            s, t = st_in[m]
            nc.scalar.activation(out=xpr[m][:], in_=xpr[m][:], func=AF.Silu,
                                 bias=t[:, 0:1], scale=s[:, 0:1])
        xp_bf = xpr  # post-silu alias
        if DEBUG:
            for m in range(NT):
                hi = min(128, CH - m * 128)
                nc.sync.dma_start(out=dbg["xp_dbg"][m * 128:m * 128 + hi, :], in_=xp_bf[m][:hi, :])

        # ---------------- phase 3: qtjdup (j-major q-tilde, head pairs) ----------------
        qtjdup = persist.tile([128, 4, NPIX], BF16, name="qtjdup")
        for c in range(8):
            qps = psum.tile([128, 4, 128], F32, tag="mm")
            for n in range(HN):
                base = 64 * (n % 2)
                for k in range(NT):
                    nc.tensor.matmul(qps[base:base + 64, n // 2, :],
                                     wqktj_sb[n][k][:],
                                     xp_bf[k][:, c * 128:(c + 1) * 128],
                                     start=(k == 0), stop=(k == NT - 1))
            nc.scalar.activation(out=qtjdup[:, :, c * 128:(c + 1) * 128],
                                 in_=qps[:], func=AF.Copy)
        if DEBUG:
            nc.sync.dma_start(out=dbg["qtj_dbg"][:, :],
                              in_=qtjdup[:].rearrange("p h x -> p (h x)"))

        # ---------------- phase 4: attention + wov, software-pipelined ----------------
        out1 = [persist.tile([128, NPIX], BF16, tag=f"o1_{m}", name=f"out1_{m}") for m in range(NT)]
        oac = [small.tile([128, 8], F32, tag=f"oac{m}", name=f"oac{m}") for m in range(NT)]
        oac2 = [persist.tile([128, 8], F32, name=f"oac2{m}") for m in range(NT)]
        prod2_t = persist.tile([128, 4, CD, D], BF16, name="prod2")

        def sim_stage(p):
            pxs = slice(p * 128, (p + 1) * 128)
            # cpf: affine+relu'd ctx slice for this tile, [j2, d, px]
            cpf = attn.tile([128, D, 128], BF16, tag="cpf", bufs=1)
            nc.scalar.activation(
                out=cpf[:], in_=cdup[:, :, pxs],
                func=AF.Relu, bias=ct[:, 0:1], scale=cs[:, 0:1])
            if with_pos:
                for d in range(D):
                    nc.vector.tensor_scalar_add(out=cpf[:, d, :], in0=cpf[:, d, :],
                                                scalar1=posdup_sb[:, d:d + 1])
            if DEBUG:
                odbg = bass.AP(tensor=dbg["ctxpf_dbg"].tensor, offset=p * 128,
                               ap=[[D * NPIX, CD], [NPIX, D], [1, 128]])
                nc.sync.dma_start(out=odbg, in_=cpf[0:CD, :, :])
            # tjd: PE transposes of cpf upper half -> [px, j, d] (2 half-d rounds)
            tjd = attn.tile([128, CD, D], BF16, tag="tjd")
            for hh in range(2):
                tp_ps = tpsum.tile([128, D // 2, CD], BF16, tag="tp")
                for d in range(D // 2):
                    nc.tensor.transpose(tp_ps[:, d, :], cpf[0:CD, hh * 16 + d, :],
                                        eye_sb[:CD, :CD])
                nc.scalar.activation(
                    out=tjd[:, :, hh * 16:(hh + 1) * 16],
                    in_=tp_ps[:].rearrange("p d j -> p j d"), func=AF.Copy)

            # sim: per head-pair j-major product; PE reduces j via mask matmuls
            expsim = attn.tile([128, D, 128], BF16, tag="expsim", bufs=1)
            es_fl = expsim[:].rearrange("p d x -> p (d x)")
            prods = attn.tile([128, 4, D, 128], BF16, tag="prodp", bufs=1,
                              name=f"prod_{p}")
            nc.vector.tensor_tensor(
                out=prods[:], in0=_bcast(cpf[:], 1, 4),
                in1=_bcast(qtjdup[:, :, pxs], 2, D), op=ALU.mult)
            for c in range(8):
                simps = mpsum.tile([128, 512], F32, tag="simps")
                for h in range(4):
                    pr = prods[:, h].rearrange("p d x -> p (d x)")
                    nc.tensor.matmul(simps[32 * h:32 * h + 32, :], mask32_sb[:],
                                     pr[:, c * 512:(c + 1) * 512],
                                     start=True, stop=True, tile_position=(0, 32 * h))
                nc.scalar.activation(out=es_fl[:, c * 512:(c + 1) * 512], in_=simps[:],
                                     func=AF.Exp)

            # transpose scores back to pixel-major [px, d, n] via psel
            tp2 = cpsum.tile([128, D, 8], BF16, tag="tp2")
            for d in range(D):
                nc.tensor.transpose(tp2[:, d, :], expsim[0:98, d, :], psel_sb[:])
            abf = attn.tile([128, HN, D], BF16, tag="abf")
            nc.scalar.activation(out=abf[:], in_=tp2[:].rearrange("p d n -> p n d"),
                                 func=AF.Copy)
            return tjd, abf

        def av_stage(p, tjd, abf):
            pxs = slice(p * 128, (p + 1) * 128)
            # normalize: a /= sum_d (exp values)
            sm = small.tile([128, 8], F32, tag="sm")
            nc.vector.tensor_reduce(out=sm[:], in_=abf[:], axis=AX.X, op=ALU.add)
            nc.vector.reciprocal(out=sm[:], in_=sm[:])
            nc.vector.tensor_tensor(
                out=abf[:], in0=abf[:], in1=_bcast(sm[:], 2, D), op=ALU.mult)
            if DEBUG:
                nc.sync.dma_start(out=dbg["a_dbg"][p * 128:(p + 1) * 128, :],
                                  in_=abf[:].rearrange("p n d -> p (n d)"))

            # ctxa in two head-halves: prod2[p, n, j, d] = tjd * a; tree-reduce d
            ctxa = attn.tile([128, 512], BF16, tag="ctxa", bufs=1)
            for hh in range(2):
                prod2 = prod2_t[:]
                nc.vector.tensor_tensor(
                    out=prod2, in0=_bcast(tjd[:], 1, 4),
                    in1=_bcast(abf[:, hh * 4:(hh + 1) * 4, :], 2, CD), op=ALU.mult)
                pv2 = prod2.rearrange("p n j d -> p (n j) d")
                w = 16
                while w >= 2:
                    nc.vector.tensor_add(pv2[:, :, 0:w], pv2[:, :, 0:w], pv2[:, :, w:2 * w])
                    w //= 2
                nc.vector.tensor_add(ctxa[:, hh * 256:(hh + 1) * 256],
                                     pv2[:, :, 0:1].squeeze(2), pv2[:, :, 1:2].squeeze(2))
            if DEBUG:
                nc.sync.dma_start(out=dbg["ctxa_dbg"][p * 128:(p + 1) * 128, :], in_=ctxa[:])

            # transpose ctxa to ch-major and project through wov into out1 cols
            cxc_ps = cpsum.tile([128, 4, 128], BF16, tag="cxc")
            for kt in range(4):
                nc.tensor.transpose(cxc_ps[:, kt, :],
                                    ctxa[:, kt * 128:(kt + 1) * 128], eye_sb[:])
            cxc = attn.tile([128, 4, 128], BF16, tag="cxcs", bufs=1)
            nc.scalar.activation(out=cxc[:], in_=cxc_ps[:], func=AF.Copy)
            for m in range(NT):
                ps = psum.tile([128, 512], F32, tag="mm")
                for k in range(4):
                    nc.tensor.matmul(ps[:, 0:128], wovt_sb[k][m][:], cxc[:, k, :],
                                     start=(k == 0), stop=(k == 3))
                nc.scalar.activation(out=out1[m][:, pxs], in_=ps[:, 0:128],
                                     func=AF.Copy, accum_out=oac[m][:, p:p + 1])
                sqj = stage.tile([128, NPIX], BF16, tag="sqjunk", bufs=1)
                nc.scalar.activation(out=sqj[:, 0:128], in_=out1[m][:, pxs],
                                     func=AF.Square, accum_out=oac2[m][:, p:p + 1])

        carry = None
        for p in range(9):
            nxt = sim_stage(p) if p < 8 else None
            if carry is not None:
                av_stage(p - 1, *carry)
            carry = nxt
        if DEBUG:
            for m in range(NT):
                hi = min(128, CH - m * 128)
                nc.sync.dma_start(out=dbg["out1_dbg"][m * 128:m * 128 + hi, :], in_=out1[m][:hi, :])

        # ---------------- phase 5: gn1+relu -> pad1 ----------------
        st1 = gn_affine_320(make_m2(out1, oac, "gn1", sq_acc=oac2), g1_g_sb, g1_b_sb, "gn1")
        pad1 = [persist.tile([128, 34, 34], BF16, tag=f"pad1_{m}", name=f"pad1_{m}") for m in range(NT)]
        for m in range(NT):
            nc.vector.memset(pad1[m][:], 0.0)
            s, t = st1[m]
            nc.scalar.activation(out=pad1[m][:, 1:33, 1:33],
                                 in_=out1[m][:].rearrange("p (h w) -> p h w", w=32),
                                 func=AF.Relu, bias=t[:, 0:1], scale=s[:, 0:1])

        # ---------------- conv helper ----------------
        def conv3x3(w_d, src_pad, name, out_bf, acc2, sq_acc=None):
            cw = [[wstream.tile([128, 9, 128], BF16, tag=f"cw_{k}_{m}",
                                name=f"{name}w_{k}_{m}") for m in range(NT)]
                  for k in range(NT)]
            for k in range(NT):
                for m in range(NT):
                    nc.sync.dma_start(out=cw[k][m][:], in_=w_d[k, m])
            for m in range(NT):
                for n in range(2):
                    r0 = n * 16
                    ps = psum.tile([128, 512], F32, tag="mm")
                    first = True
                    for tap in range(9):
                        dy, dx = tap // 3, tap % 3
                        for k in range(NT):
                            nc.tensor.matmul(
                                ps[:], cw[k][m][:, tap, :],
                                src_pad[k][:, r0 + dy:r0 + dy + 16, dx:dx + 32],
                                start=first, stop=(tap == 8 and k == NT - 1))
                            first = False
                    nc.scalar.activation(out=out_bf[m][:, n * 512:(n + 1) * 512], in_=ps[:],
                                         func=AF.Copy, accum_out=acc2[m][:, n:n + 1])
                if sq_acc is not None:
                    sqj = stage.tile([128, NPIX], BF16, tag="sqjunk", bufs=1)
                    nc.scalar.activation(out=sqj[:], in_=out_bf[m][:],
                                         func=AF.Square, accum_out=sq_acc[m][:, 0:1])

        y2 = [persist.tile([128, NPIX], BF16, tag=f"o1_{m}", name=f"y2_{m}") for m in range(NT)]
        yac = [small.tile([128, 2], F32, tag=f"yac{m}", name=f"yac{m}") for m in range(NT)]
        yac2 = [persist.tile([128, 1], F32, name=f"yac2{m}") for m in range(NT)]
        conv3x3(c1_d, pad1, "c1", y2, yac, sq_acc=yac2)
        st2 = gn_affine_320(make_m2(y2, yac, "gn2", sq_acc=yac2), g2_g_sb, g2_b_sb, "gn2")
        pad2 = [persist.tile([128, 34, 34], BF16, tag=f"pad1_{m}", name=f"pad2_{m}") for m in range(NT)]
        for m in range(NT):
            nc.vector.memset(pad2[m][:], 0.0)
            s, t = st2[m]
            nc.scalar.activation(out=pad2[m][:, 1:33, 1:33],
                                 in_=y2[m][:].rearrange("p (h w) -> p h w", w=32),
                                 func=AF.Relu, bias=t[:, 0:1], scale=s[:, 0:1])

        # ---------------- conv2 + residual (identity matmul) ----------------
        cw2 = [[wstream.tile([128, 9, 128], BF16, tag=f"cw_{k}_{m}",
                             name=f"c2w_{k}_{m}") for m in range(NT)]
               for k in range(NT)]
        for k in range(NT):
            for m in range(NT):
                nc.sync.dma_start(out=cw2[k][m][:], in_=c2_d[k, m])
        for m in range(NT):
            hi = min(128, CH - m * 128)
            for n in range(2):
                r0 = n * 16
                ps = psum.tile([128, 512], F32, tag="mm")
                nc.tensor.matmul(ps[:], eye_sb[:], xbf_sb[m][:, n * 512:(n + 1) * 512],
                                 start=True, stop=False)
                for tap in range(9):
                    dy, dx = tap // 3, tap % 3
                    for k in range(NT):
                        nc.tensor.matmul(
                            ps[:], cw2[k][m][:, tap, :],
                            pad2[k][:, r0 + dy:r0 + dy + 16, dx:dx + 32],
                            start=False, stop=(tap == 8 and k == NT - 1))
                fin = stage.tile([128, 512], F32, tag="fin", bufs=1)
                nc.scalar.activation(out=fin[:], in_=ps[:], func=AF.Copy)
                nc.sync.dma_start(out=y_d[m * 128:m * 128 + hi, n * 512:(n + 1) * 512],
                                  in_=fin[:hi, :])
        es.close()

    nc.compile()
    return nc


_PROGS = {}
_LAST_RESULTS = None
_LAST_EXEC_NS = None


def _get_prog(with_pos):
    if with_pos not in _PROGS:
        _PROGS[with_pos] = build_program(with_pos)
    return _PROGS[with_pos]


def _prep_host(inputs, with_pos):
    """Precompute folded weights; returns the common (weight) part of in_map."""
    f32 = np.float32
    bf16 = ml_dtypes.bfloat16
    w_in = np.asarray(inputs["w_in"], f32)
    wq = np.asarray(inputs["wq"], f32)
    wk = np.asarray(inputs["wk"], f32)
    wv = np.asarray(inputs["wv"], f32)
    wout = np.asarray(inputs["w_attn_out"], f32)
    pos = np.asarray(inputs["pos_emb"], f32)
    scale = HD ** -0.5

    def pad_to(a, shape):
        out = np.zeros(shape, a.dtype)
        out[tuple(slice(0, s) for s in a.shape)] = a
        return out

    def tile_km(mat_t, kt, mt):  # mat_t: [K, M] -> [kt, mt, 128, 128]
        p = pad_to(mat_t, (kt * 128, mt * 128))
        return np.ascontiguousarray(
            p.reshape(kt, 128, mt, 128).transpose(0, 2, 1, 3))

    w_in_tiles = tile_km(w_in.T, NT, NT).astype(bf16)

    # wqktj: per head, wqk_n^T [320->384, 64] tiled into [HN, NT, 128, 64]
    wqktj = np.zeros((HN, NT, 128, CD), f32)
    for n in range(HN):
        wqk_n = scale * (wk[n * HD:(n + 1) * HD, :].T @ wq[n * HD:(n + 1) * HD, :])
        wqktj[n] = pad_to(wqk_n.T, (NT * 128, CD)).reshape(NT, 128, CD)
    wqktj = wqktj.astype(bf16)

    wov = np.concatenate(
        [wout[:, n * HD:(n + 1) * HD] @ wv[n * HD:(n + 1) * HD, :]
         for n in range(HN)], axis=1)          # [320, 512]
    wov_tiles = tile_km(wov.T, 4, NT).astype(bf16)

    def conv_tiles(w):  # [o, i, 3, 3] -> [kt, mt, 128, 9, 128] (tap-interleaved lhsT)
        taps = np.stack([tile_km(np.ascontiguousarray(w[:, :, t // 3, t % 3].T), NT, NT)
                         for t in range(9)], axis=0)  # [9, kt, mt, 128, 128]
        return np.ascontiguousarray(taps.transpose(1, 2, 3, 0, 4)).astype(bf16)

    gsel8 = np.zeros((NT * 128, 8), f32)
    gselT = np.zeros((8, NT * 128), f32)
    for g in range(8):
        gsel8[g * 40:(g + 1) * 40, g] = 1.0 / (40 * NPIX)
        gselT[g, g * 40:(g + 1) * 40] = 1.0
    # ctx group aggregation (8 groups of 8 within 64ch, dup'd); bn_aggr
    # supplies per-channel (mean, E2) so entries are 1/8.
    g2ctx = np.zeros((128, 128), f32)
    for h in range(2):
        for g in range(8):
            sl = slice(h * 64 + g * 8, h * 64 + (g + 1) * 8)
            g2ctx[sl, sl] = 1.0 / 8

    mask32 = np.zeros((128, 32), f32)
    mask32[0:64, 0] = 1.0
    mask32[64:128, 1] = 1.0
    psel = np.zeros((98, 8), f32)
    for n in range(HN):
        psel[32 * (n // 2) + (n % 2), n] = 1.0

    def col(v):
        return pad_to(np.asarray(v, f32).reshape(-1, 1), (384, 1))

    wctx_t = np.ascontiguousarray(np.asarray(inputs["w_ctx"], f32).T)

    common = {
        "w_in_t": w_in_tiles,
        "b_in": col(inputs["b_in"]),
        "gin_g": col(inputs["gn_in_g"]), "gin_b": col(inputs["gn_in_b"]),
        "wctxdup": np.tile(np.concatenate([wctx_t, wctx_t], axis=1), (2, 1)).astype(bf16),
        "gctx_g": np.tile(np.asarray(inputs["gn_ctx_g"], f32), 2).reshape(128, 1),
        "gctx_b": np.tile(np.asarray(inputs["gn_ctx_b"], f32), 2).reshape(128, 1),
        "wqktj": wqktj,
        "wovt": wov_tiles,
        "g1_g": col(inputs["gn1_g"]), "g1_b": col(inputs["gn1_b"]),
        "g2_g": col(inputs["gn2_g"]), "g2_b": col(inputs["gn2_b"]),
        "conv1_t": conv_tiles(np.asarray(inputs["conv1_w"], f32)),
        "conv2_t": conv_tiles(np.asarray(inputs["conv2_w"], f32)),
        "gsel8": np.ascontiguousarray(gsel8.reshape(NT, 128, 8)),
        "gselT": np.ascontiguousarray(gselT.reshape(8, NT, 128).transpose(1, 0, 2)),
        "g2ctx": g2ctx,
        "eye": np.eye(128, dtype=bf16),
        "mask32": mask32.astype(bf16),
        "psel": psel.astype(bf16),
    }
    if with_pos:
        # posT [j, d] duplicated on both partition halves, f32
        posT = np.ascontiguousarray(pos.T)  # [cdim=64, depth=32]
        common["pos_dup"] = np.concatenate([posT, posT], axis=0)
    return common


def kernel(**inputs):
    with_pos = bool(np.any(np.asarray(inputs["pos_emb"])))
    nc = _get_prog(with_pos)
    common = _prep_host(inputs, with_pos)
    x = np.asarray(inputs["x"], np.float32)      # [6, 320, 32, 32]
    ctx = np.asarray(inputs["context"], np.float32)  # [6, 64, 32, 32, 32]
    b = x.shape[0]
    in_maps = []
    for core in range(8):
        s = core if core < b else core - b
        m = dict(common)
        xs = np.zeros((384, NPIX), np.float32)
        xs[:CH] = x[s].reshape(CH, NPIX)
        m["x_bf"] = xs.astype(ml_dtypes.bfloat16)
        m["ctxin"] = np.ascontiguousarray(
            ctx[s].reshape(CD, D * NPIX)).astype(ml_dtypes.bfloat16)
        in_maps.append(m)
    trace = bool(int(os.environ.get("DT_TRACE", "0")))
    kw = {}
    if trace:
        import sys
        import types
        try:
            import antenv.axon_hooks  # noqa: F401
        except ImportError:
            from trn_agent_boot.trn_boot import _ntff_profile_via_ctypes
            m = types.ModuleType("antenv.axon_hooks")
            _h = _ntff_profile_via_ctypes("/opt/axon/libaxon_pjrt.so")
            m.get_axon_ntff_profile_hook = lambda: _h
            sys.modules["antenv.axon_hooks"] = m
        kw = dict(trace=True, tmpdir=os.environ.get("DT_TRACE_DIR") or None)
    res = run_bass_kernel_spmd(nc, in_maps, list(range(8)), **kw)
    global _LAST_RESULTS, _LAST_EXEC_NS
    _LAST_RESULTS = res.results
    _LAST_EXEC_NS = res.exec_time_ns
    if trace:
        print(f"HW exec time: {res.exec_time_ns} ns")
    out = np.stack([res.results[s]["y"] for s in range(b)], axis=0)
    return out.reshape(b, CH, 32, 32).astype(np.float32)


if __name__ == "__main__":
    pass


# revision 22
# speedup vs baseline: 1.0429x; 1.0429x over previous
"""DepthTransformer Trainium2 kernel.

Data-parallel over the batch dim b=6 across 8 cores (cores 6,7 duplicate;
host discards). Per core, one full sample:

  xp   = silu(gn_in(W_in @ x + b_in))                      [320, 1024]
  cdup = W_ctx @ ctx duplicated on 128 partitions (j-dup), raw; GN stats
         via DVE bn_stats during the PSUM evacs (scalar engine).
  Attention, software-pipelined over 8 pixel tiles (the ctxa/wov stage
  lags the sim/softmax stage by one tile so the DVE never waits on the
  PE->scalar softmax chain):
    cpf  = relu(cs*cdup+ct) per tile;  qtjdup[j2, h2, px] holds the
      j-major folded q (scale*wk^T wq @ xp) with head pairs on partition
      halves (even head on j-rows 0-63, odd on 64-127).
    sim: one DVE 2x-bf16 product cpf*qtjdup per tile [128,(4,32,128)];
      the j-reduction runs on the PE via pair-mask matmuls at four
      tile_position col-groups (rows 32h+{0,1} of chunked PSUM); exp()
      is fused into the PSUM evac (no max subtraction: |sim| << 1 for
      this net); PE transposes with a selection matrix (psel) return the
      scores to pixel-major [px, (d, n)]; softmax denominator on DVE.
    ctxa: pixel-major prod2 (tjd * a broadcast) + in-place tree
      reduction over d on DVE, in two head-halves; tjd comes from PE
      transposes of cpf.  wov projection via PE transposes of ctxa.
  y1 = relu(gn1(out1)); y2 = relu(gn2(conv3x3(y1, w1)))
  y  = conv3x3(y2, w2) + x   (residual via identity matmul in PSUM)

GroupNorm stats: evac accum_out columns for sums; sumsq via scalar
Square+accum passes overlapped with attention (out1) and conv1 (y2),
DVE square for xp, bn_stats for ctx.  Group aggregation via small
matmuls with exact f32 weights.  Weight DMAs not needed before the
attention phase are issued after the ctx stream.
"""

import os
import numpy as np
import ml_dtypes

import concourse.bass as bass
import concourse.bacc as bacc
import concourse.tile as tile
from concourse import mybir
from concourse.bass_utils import run_bass_kernel_spmd

F32 = mybir.dt.float32
BF16 = mybir.dt.bfloat16
AF = mybir.ActivationFunctionType
ALU = mybir.AluOpType
AX = mybir.AxisListType

HN, HD, CD, D = 8, 40, 64, 32
CH = HN * HD          # 320
NPIX = 1024           # 32*32
NT = 3                # channel tiles of 128 for 320 (128,128,64 padded to 384)
EPS = 1e-5
DEBUG = bool(int(os.environ.get("DT_DEBUG", "0")))


def _bcast(ap, axis, n):
    """Insert a step-0 broadcast dim of size n at free-dim position `axis`."""
    return ap.unsqueeze(axis).broadcast_to(
        tuple(ap.shape[:axis]) + (n,) + tuple(ap.shape[axis:]))


def build_program(with_pos):
    nc = bacc.Bacc("TRN2", target_bir_lowering=False, debug=False)

    def inp(name, shape, dt=F32):
        return nc.dram_tensor(name, shape, dt, kind="ExternalInput").ap()

    xbf_d = inp("x_bf", [384, NPIX], BF16)
    ctx_d = inp("ctxin", [CD, D * NPIX], BF16)
    w_in_t = inp("w_in_t", [NT, NT, 128, 128], BF16)
    b_in = inp("b_in", [384, 1])
    gin_g = inp("gin_g", [384, 1])
    gin_b = inp("gin_b", [384, 1])
    wctxdup_d = inp("wctxdup", [128, 128], BF16)
    gctx_g = inp("gctx_g", [128, 1])
    gctx_b = inp("gctx_b", [128, 1])
    wqktj_d = inp("wqktj", [HN, NT, 128, CD], BF16)
    if with_pos:
        pos_dj = inp("pos_dup", [128, D])   # posT dup'd: [j2, d] f32
    wovt_d = inp("wovt", [4, NT, 128, 128], BF16)
    g1_g = inp("g1_g", [384, 1])
    g1_b = inp("g1_b", [384, 1])
    g2_g = inp("g2_g", [384, 1])
    g2_b = inp("g2_b", [384, 1])
    c1_d = inp("conv1_t", [NT, NT, 128, 9, 128], BF16)
    c2_d = inp("conv2_t", [NT, NT, 128, 9, 128], BF16)
    gsel8_d = inp("gsel8", [NT, 128, 8])
    gselT_d = inp("gselT", [NT, 8, 128])
    g2c_d = inp("g2ctx", [128, 128])
    eye_d = inp("eye", [128, 128], BF16)
    mask32_d = inp("mask32", [128, 32], BF16)
    psel_d = inp("psel", [98, 8], BF16)

    y_d = nc.dram_tensor("y", [CH, NPIX], F32, kind="ExternalOutput").ap()

    dbg = {}
    if DEBUG:
        for nm, shape, dt in [
            ("xp_dbg", [CH, NPIX], BF16),
            ("ctxpf_dbg", [CD, D * NPIX], BF16),
            ("qtj_dbg", [128, 4 * NPIX], BF16),
            ("a_dbg", [NPIX, 256], BF16),
            ("ctxa_dbg", [NPIX, 512], BF16),
            ("out1_dbg", [CH, NPIX], BF16),
        ]:
            dbg[nm] = nc.dram_tensor(nm, shape, dt, kind="ExternalOutput").ap()

    with tile.TileContext(nc) as tc:
        from contextlib import ExitStack

        es = ExitStack()
        persist = es.enter_context(tc.tile_pool(name="persist", bufs=1))
        wpool = es.enter_context(tc.tile_pool(name="wpool", bufs=1))
        wstream = es.enter_context(tc.tile_pool(name="wstream", bufs=1))
        stage = es.enter_context(tc.tile_pool(name="stage", bufs=3))
        small = es.enter_context(tc.tile_pool(name="small", bufs=2))
        attn = es.enter_context(tc.tile_pool(name="attn", bufs=2))
        psum = es.enter_context(tc.tile_pool(name="psum", bufs=2, space="PSUM"))
        spsum = es.enter_context(tc.tile_pool(name="spsum", bufs=1, space="PSUM"))
        tpsum = es.enter_context(tc.tile_pool(name="tpsum", bufs=1, space="PSUM"))
        cpsum = es.enter_context(tc.tile_pool(name="cpsum", bufs=1, space="PSUM"))
        mpsum = es.enter_context(tc.tile_pool(name="mpsum", bufs=2, space="PSUM"))

        # ---------------- load x + proj_in weights first ----------------
        _wn = [0]
        def load_w(pool, src, shape, dt=F32, tag=None):
            _wn[0] += 1
            t = pool.tile(shape, dt, tag=tag or "", name=f"w{_wn[0]}")
            nc.sync.dma_start(out=t[:], in_=src)
            return t

        xbf_sb = [persist.tile([128, NPIX], BF16, name=f"xb_{t}") for t in range(NT)]
        for t in range(NT):
            nc.sync.dma_start(out=xbf_sb[t][:], in_=xbf_d[t * 128:(t + 1) * 128, :])
        win_sb = [[load_w(wpool, w_in_t[k, m], [128, 128], BF16,
                          tag=f"wsh{k * NT + m}") for m in range(NT)]
                  for k in range(NT)]
        bin_sb = [load_w(wpool, b_in[m * 128:(m + 1) * 128], [128, 1]) for m in range(NT)]

        wctxdup_sb = load_w(wpool, wctxdup_d, [128, 128], BF16)
        eye_sb = load_w(wpool, eye_d, [128, 128], BF16)

        def load_vec(src):
            return [load_w(wpool, src[m * 128:(m + 1) * 128], [128, 1]) for m in range(NT)]

        gin_g_sb, gin_b_sb = load_vec(gin_g), load_vec(gin_b)
        gctx_g_sb = load_w(wpool, gctx_g, [128, 1])
        gctx_b_sb = load_w(wpool, gctx_b, [128, 1])

        eps_sb = wpool.tile([128, 1], F32)
        nc.vector.memset(eps_sb[:], EPS)

        if with_pos:
            posdup_sb = load_w(wpool, pos_dj, [128, D])

        def load_deferred():
            # weights not needed until phase 3+: issue their DMAs after the
            # ctx stream so input chunks aren't queued behind them
            w = {}
            w["wqktj"] = [[load_w(wpool, wqktj_d[n, k], [128, CD], BF16)
                           for k in range(NT)] for n in range(HN)]
            w["wovt"] = [[load_w(wpool, wovt_d[k, m], [128, 128], BF16,
                                 tag=f"wsh{k * NT + m}" if k * NT + m < 9 else None)
                          for m in range(NT)] for k in range(4)]
            w["gsel8"] = [load_w(wpool, gsel8_d[k], [128, 8]) for k in range(NT)]
            w["gselT"] = [load_w(wpool, gselT_d[m], [8, 128]) for m in range(NT)]
            w["g2c"] = load_w(wpool, g2c_d, [128, 128])
            w["mask32"] = load_w(wpool, mask32_d, [128, 32], BF16)
            w["psel"] = load_w(wpool, psel_d, [98, 8], BF16)
            w["g1"] = (load_vec(g1_g), load_vec(g1_b))
            w["g2"] = (load_vec(g2_g), load_vec(g2_b))
            return w

        # =========== GN finalize (320-channel, 8 groups of 40) ===========
        def gn_affine_320(m2, gamma_sb, beta_sb, name):
            gp8 = spsum.tile([128, 2], F32, tag="gnps")
            for k in range(NT):
                nc.tensor.matmul(gp8[:8, :], gsel8_sb[k][:], m2[k][:],
                                 start=(k == 0), stop=(k == NT - 1))
            g8 = small.tile([8, 2], F32, tag=f"{name}_g8")
            nc.vector.tensor_copy(g8[:], gp8[:8, :])
            s_t = []
            for m in range(NT):
                gp = spsum.tile([128, 2], F32, tag="gnps")
                nc.tensor.matmul(gp[:], gselT_sb[m][:], g8[:], start=True, stop=True)
                gs = small.tile([128, 2], F32, tag=f"{name}_gs{m}")
                nc.vector.tensor_copy(gs[:], gp[:])
                s = small.tile([128, 1], F32, tag=f"{name}_s{m}")
                tt = small.tile([128, 1], F32, tag=f"{name}_t{m}")
                vt = small.tile([128, 1], F32, tag=f"{name}_v")
                nc.vector.tensor_mul(vt[:], gs[:, 0:1], gs[:, 0:1])
                nc.vector.tensor_sub(vt[:], gs[:, 1:2], vt[:])
                nc.scalar.activation(out=vt[:], in_=vt[:], func=AF.Sqrt, bias=eps_sb[:, 0:1])
                nc.vector.reciprocal(out=vt[:], in_=vt[:])
                nc.vector.tensor_mul(s[:], gamma_sb[m][:], vt[:])
                nc.vector.tensor_mul(tt[:], gs[:, 0:1], s[:])
                nc.vector.tensor_sub(tt[:], beta_sb[m][:], tt[:])
                s_t.append((s, tt))
            return s_t

        # build (sum, sumsq) m2 tiles from evac accum cols + one square pass
        def make_m2(src_bf, acc, name, sq_acc=None):
            m2 = []
            for m in range(NT):
                t = small.tile([128, 2], F32, tag=f"{name}_m2{m}", name=f"{name}_m2{m}")
                nc.vector.tensor_reduce(out=t[:, 0:1], in_=acc[m][:], axis=AX.X, op=ALU.add)
                if sq_acc is not None:
                    nc.vector.tensor_reduce(out=t[:, 1:2], in_=sq_acc[m][:], axis=AX.X,
                                            op=ALU.add)
                else:
                    sq = stage.tile([128, NPIX], BF16, tag="sqjunk", bufs=1)
                    nc.vector.scalar_tensor_tensor(
                        out=sq[:], in0=src_bf[m][:], scalar=1.0, in1=src_bf[m][:],
                        op0=ALU.mult, op1=ALU.mult, accum_out=t[:, 1:2])
                m2.append(t)
            return m2

        # ---------------- phase 1: proj_in ----------------
        xpr = [persist.tile([128, NPIX], BF16, tag=f"o1_{t}", name=f"xpr_{t}") for t in range(NT)]
        xac = [small.tile([128, 2], F32, tag=f"xac{m}", name=f"xac{m}") for m in range(NT)]
        for m in range(NT):
            for n in range(2):
                ps = psum.tile([128, 512], F32, tag="mm")
                for k in range(NT):
                    nc.tensor.matmul(ps[:], win_sb[k][m][:], xbf_sb[k][:, n * 512:(n + 1) * 512],
                                     start=(k == 0), stop=(k == NT - 1))
                nc.scalar.activation(out=xpr[m][:, n * 512:(n + 1) * 512], in_=ps[:],
                                     func=AF.Identity, bias=bin_sb[m][:, 0:1],
                                     accum_out=xac[m][:, n:n + 1])

        # ---------------- phase 2: cdup (dup ctx proj, raw) + DVE stats ----------------
        cdup = persist.tile([128, D, NPIX], BF16, name="cdup")
        cdup_fl = cdup[:].rearrange("j d p -> j (d p)")
        NCH = 64
        cst = persist.tile([128, NCH, 6], F32, name="cst")
        for cc in range(NCH // 2):
            cin = stage.tile([128, 512], BF16, tag="ctxin", bufs=3)
            nc.sync.dma_start(out=cin[0:CD, :], in_=ctx_d[:, (2 * cc) * 512:(2 * cc + 1) * 512])
            nc.sync.dma_start(out=cin[CD:128, :], in_=ctx_d[:, (2 * cc + 1) * 512:(2 * cc + 2) * 512])
            for half in range(2):
                c = 2 * cc + half
                ps = psum.tile([128, 512], F32, tag="mm")
                nc.tensor.matmul(ps[:], wctxdup_sb[half * CD:(half + 1) * CD, :],
                                 cin[half * CD:(half + 1) * CD, :], start=True, stop=True)
                nc.scalar.activation(out=cdup_fl[:, c * 512:(c + 1) * 512], in_=ps[:],
                                     func=AF.Copy)
                nc.vector.bn_stats(out=cst[:, c, :], in_=cdup_fl[:, c * 512:(c + 1) * 512])
        wdef = load_deferred()
        wqktj_sb, wovt_sb = wdef["wqktj"], wdef["wovt"]
        gsel8_sb, gselT_sb, g2c_sb = wdef["gsel8"], wdef["gselT"], wdef["g2c"]
        mask32_sb, psel_sb = wdef["mask32"], wdef["psel"]
        g1_g_sb, g1_b_sb = wdef["g1"]
        g2_g_sb, g2_b_sb = wdef["g2"]
        cmv = small.tile([128, 2], F32)
        nc.vector.bn_aggr(out=cmv[:], in_=cst[:])
        cm2 = small.tile([128, 2], F32)
        nc.vector.tensor_copy(cm2[:, 0:1], cmv[:, 0:1])
        nc.vector.tensor_mul(cm2[:, 1:2], cmv[:, 0:1], cmv[:, 0:1])
        nc.vector.tensor_add(cm2[:, 1:2], cm2[:, 1:2], cmv[:, 1:2])
        # group aggregate: g2c entries are 1/8 -> cgs = (mean, E2)
        cgp = spsum.tile([128, 2], F32, tag="gnps")
        nc.tensor.matmul(cgp[:], g2c_sb[:], cm2[:], start=True, stop=True)
        cgs = small.tile([128, 2], F32)
        nc.vector.tensor_copy(cgs[:], cgp[:])
        cs = small.tile([128, 1], F32)
        ct = small.tile([128, 1], F32)
        cv = small.tile([128, 1], F32)
        nc.vector.tensor_mul(cv[:], cgs[:, 0:1], cgs[:, 0:1])
        nc.vector.tensor_sub(cv[:], cgs[:, 1:2], cv[:])
        nc.scalar.activation(out=cv[:], in_=cv[:], func=AF.Sqrt, bias=eps_sb[:, 0:1])
        nc.vector.reciprocal(out=cv[:], in_=cv[:])
        nc.vector.tensor_mul(cs[:], gctx_g_sb[:], cv[:])
        nc.vector.tensor_mul(ct[:], cgs[:, 0:1], cs[:])
        nc.vector.tensor_sub(ct[:], gctx_b_sb[:], ct[:])
        if with_pos:
            # ctpos[j2, d] = ct[j2] + posT[j, d] (bias per (partition, d-slice))
            ctpos = small.tile([128, D], F32, name="ctpos")
            nc.vector.tensor_scalar_add(out=ctpos[:], in0=posdup_sb[:],
                                        scalar1=ct[:, 0:1])

        # ---------------- phase 1b: gn_in finalize + silu (in place) ----------------
        st_in = gn_affine_320(make_m2(xpr, xac, "gin"), gin_g_sb, gin_b_sb, "gin")
        for m in range(NT):
            s, t = st_in[m]
            nc.scalar.activation(out=xpr[m][:], in_=xpr[m][:], func=AF.Silu,
                                 bias=t[:, 0:1], scale=s[:, 0:1])
        xp_bf = xpr  # post-silu alias
        if DEBUG:
            for m in range(NT):
                hi = min(128, CH - m * 128)
                nc.sync.dma_start(out=dbg["xp_dbg"][m * 128:m * 128 + hi, :], in_=xp_bf[m][:hi, :])

        # ---------------- phase 3: qtjdup (j-major q-tilde, head pairs) ----------------
        qtjdup = persist.tile([128, 4, NPIX], BF16, name="qtjdup")
        for c in range(8):
            qps = psum.tile([128, 4, 128], F32, tag="mm")
            for n in range(HN):
                base = 64 * (n % 2)
                for k in range(NT):
                    nc.tensor.matmul(qps[base:base + 64, n // 2, :],
                                     wqktj_sb[n][k][:],
                                     xp_bf[k][:, c * 128:(c + 1) * 128],
                                     start=(k == 0), stop=(k == NT - 1))
            nc.scalar.activation(out=qtjdup[:, :, c * 128:(c + 1) * 128],
                                 in_=qps[:], func=AF.Copy)
        if DEBUG:
            nc.sync.dma_start(out=dbg["qtj_dbg"][:, :],
                              in_=qtjdup[:].rearrange("p h x -> p (h x)"))

        # ---------------- phase 4: attention + wov, software-pipelined ----------------
        out1 = [persist.tile([128, NPIX], BF16, tag=f"o1_{m}", name=f"out1_{m}") for m in range(NT)]
        oac = [small.tile([128, 8], F32, tag=f"oac{m}", name=f"oac{m}") for m in range(NT)]
        oac2 = [persist.tile([128, 8], F32, name=f"oac2{m}") for m in range(NT)]
        prod2_t = persist.tile([128, 4, CD, D], BF16, name="prod2")

        def sim_stage(p):
            pxs = slice(p * 128, (p + 1) * 128)
            # cpf: affine+relu'd ctx slice for this tile, [j2, d, px]
            cpf = attn.tile([128, D, 128], BF16, tag="cpf", bufs=1)
            nc.scalar.activation(
                out=cpf[:], in_=cdup[:, :, pxs],
                func=AF.Relu, bias=ct[:, 0:1], scale=cs[:, 0:1])
            if with_pos:
                for d in range(D):
                    nc.vector.tensor_scalar_add(out=cpf[:, d, :], in0=cpf[:, d, :],
                                                scalar1=posdup_sb[:, d:d + 1])
            if DEBUG:
                odbg = bass.AP(tensor=dbg["ctxpf_dbg"].tensor, offset=p * 128,
                               ap=[[D * NPIX, CD], [NPIX, D], [1, 128]])
                nc.sync.dma_start(out=odbg, in_=cpf[0:CD, :, :])
            # tjd: PE transposes of cpf upper half -> [px, j, d] (2 half-d rounds)
            tjd = attn.tile([128, CD, D], BF16, tag="tjd")
            for hh in range(2):
                tp_ps = tpsum.tile([128, D // 2, CD], BF16, tag="tp")
                for d in range(D // 2):
                    nc.tensor.transpose(tp_ps[:, d, :], cpf[0:CD, hh * 16 + d, :],
                                        eye_sb[:CD, :CD])
                nc.scalar.activation(
                    out=tjd[:, :, hh * 16:(hh + 1) * 16],
                    in_=tp_ps[:].rearrange("p d j -> p j d"), func=AF.Copy)

            # sim: per head-pair j-major product; PE reduces j via mask matmuls
            expsim = attn.tile([128, D, 128], BF16, tag="expsim", bufs=1)
            es_fl = expsim[:].rearrange("p d x -> p (d x)")
            prods = attn.tile([128, 4, D, 128], BF16, tag="prodp", bufs=1,
                              name=f"prod_{p}")
            nc.vector.tensor_tensor(
                out=prods[:], in0=_bcast(cpf[:], 1, 4),
                in1=_bcast(qtjdup[:, :, pxs], 2, D), op=ALU.mult)
            for c in range(8):
                simps = mpsum.tile([128, 512], F32, tag="simps")
                for h in range(4):
                    pr = prods[:, h].rearrange("p d x -> p (d x)")
                    nc.tensor.matmul(simps[32 * h:32 * h + 32, :], mask32_sb[:],
                                     pr[:, c * 512:(c + 1) * 512],
                                     start=True, stop=True, tile_position=(0, 32 * h))
                nc.scalar.activation(out=es_fl[:, c * 512:(c + 1) * 512], in_=simps[:],
                                     func=AF.Exp)

            # transpose scores back to pixel-major [px, d, n] via psel
            tp2 = cpsum.tile([128, D, 8], BF16, tag="tp2")
            for d in range(D):
                nc.tensor.transpose(tp2[:, d, :], expsim[0:98, d, :], psel_sb[:])
            abf = attn.tile([128, HN, D], BF16, tag="abf")
            nc.scalar.activation(out=abf[:], in_=tp2[:].rearrange("p d n -> p n d"),
                                 func=AF.Copy)
            return tjd, abf

        def av_stage(p, tjd, abf):
            pxs = slice(p * 128, (p + 1) * 128)
            # normalize: a /= sum_d (exp values)
            sm = small.tile([128, 8], F32, tag="sm")
            nc.vector.tensor_reduce(out=sm[:], in_=abf[:], axis=AX.X, op=ALU.add)
            nc.vector.reciprocal(out=sm[:], in_=sm[:])
            nc.vector.tensor_tensor(
                out=abf[:], in0=abf[:], in1=_bcast(sm[:], 2, D), op=ALU.mult)
            if DEBUG:
                nc.sync.dma_start(out=dbg["a_dbg"][p * 128:(p + 1) * 128, :],
                                  in_=abf[:].rearrange("p n d -> p (n d)"))

            # ctxa in two head-halves: prod2[p, n, j, d] = tjd * a; tree-reduce d
            ctxa = attn.tile([128, 512], BF16, tag="ctxa", bufs=1)
            for hh in range(2):
                prod2 = prod2_t[:]
                nc.vector.tensor_tensor(
                    out=prod2, in0=_bcast(tjd[:], 1, 4),
                    in1=_bcast(abf[:, hh * 4:(hh + 1) * 4, :], 2, CD), op=ALU.mult)
                pv2 = prod2.rearrange("p n j d -> p (n j) d")
                w = 16
                while w >= 2:
                    nc.vector.tensor_add(pv2[:, :, 0:w], pv2[:, :, 0:w], pv2[:, :, w:2 * w])
                    w //= 2
                nc.vector.tensor_add(ctxa[:, hh * 256:(hh + 1) * 256],
                                     pv2[:, :, 0:1].squeeze(2), pv2[:, :, 1:2].squeeze(2))
            if DEBUG:
                nc.sync.dma_start(out=dbg["ctxa_dbg"][p * 128:(p + 1) * 128, :], in_=ctxa[:])

            # transpose ctxa to ch-major and project through wov into out1 cols
            cxc_ps = cpsum.tile([128, 4, 128], BF16, tag="cxc")
            for kt in range(4):
                nc.tensor.transpose(cxc_ps[:, kt, :],
                                    ctxa[:, kt * 128:(kt + 1) * 128], eye_sb[:])
            cxc = attn.tile([128, 4, 128], BF16, tag="cxcs", bufs=1)
            nc.scalar.activation(out=cxc[:], in_=cxc_ps[:], func=AF.Copy)
            for m in range(NT):
                ps = psum.tile([128, 512], F32, tag="mm")
                for k in range(4):
                    nc.tensor.matmul(ps[:, 0:128], wovt_sb[k][m][:], cxc[:, k, :],
                                     start=(k == 0), stop=(k == 3))
                nc.scalar.activation(out=out1[m][:, pxs], in_=ps[:, 0:128],
                                     func=AF.Copy, accum_out=oac[m][:, p:p + 1])
                sqj = stage.tile([128, NPIX], BF16, tag="sqjunk", bufs=1)
                nc.scalar.activation(out=sqj[:, 0:128], in_=out1[m][:, pxs],
                                     func=AF.Square, accum_out=oac2[m][:, p:p + 1])

        carry = None
        for p in range(9):
            nxt = sim_stage(p) if p < 8 else None
            if carry is not None:
                av_stage(p - 1, *carry)
            carry = nxt
        if DEBUG:
            for m in range(NT):
                hi = min(128, CH - m * 128)
                nc.sync.dma_start(out=dbg["out1_dbg"][m * 128:m * 128 + hi, :], in_=out1[m][:hi, :])

        # ---------------- phase 5: gn1+relu -> pad1 ----------------
        st1 = gn_affine_320(make_m2(out1, oac, "gn1", sq_acc=oac2), g1_g_sb, g1_b_sb, "gn1")
        pad1 = [persist.tile([128, 34, 34], BF16, tag=f"pad1_{m}", name=f"pad1_{m}") for m in range(NT)]
        for m in range(NT):
            nc.vector.memset(pad1[m][:], 0.0)
            s, t = st1[m]
            nc.scalar.activation(out=pad1[m][:, 1:33, 1:33],
                                 in_=out1[m][:].rearrange("p (h w) -> p h w", w=32),
                                 func=AF.Relu, bias=t[:, 0:1], scale=s[:, 0:1])

        # ---------------- conv helper ----------------
        def conv3x3(w_d, src_pad, name, out_bf, acc2, sq_acc=None):
            cw = [[wstream.tile([128, 9, 128], BF16, tag=f"cw_{k}_{m}",
                                name=f"{name}w_{k}_{m}") for m in range(NT)]
                  for k in range(NT)]
            for k in range(NT):
                for m in range(NT):
                    nc.sync.dma_start(out=cw[k][m][:], in_=w_d[k, m])
            for m in range(NT):
                for n in range(2):
                    r0 = n * 16
                    ps = psum.tile([128, 512], F32, tag="mm")
                    first = True
                    for tap in range(9):
                        dy, dx = tap // 3, tap % 3
                        for k in range(NT):
                            nc.tensor.matmul(
                                ps[:], cw[k][m][:, tap, :],
                                src_pad[k][:, r0 + dy:r0 + dy + 16, dx:dx + 32],
                                start=first, stop=(tap == 8 and k == NT - 1))
                            first = False
                    nc.scalar.activation(out=out_bf[m][:, n * 512:(n + 1) * 512], in_=ps[:],
                                         func=AF.Copy, accum_out=acc2[m][:, n:n + 1])
                if sq_acc is not None:
                    sqj = stage.tile([128, NPIX], BF16, tag="sqjunk", bufs=1)
                    nc.scalar.activation(out=sqj[:], in_=out_bf[m][:],
                                         func=AF.Square, accum_out=sq_acc[m][:, 0:1])

        y2 = [persist.tile([128, NPIX], BF16, tag=f"o1_{m}", name=f"y2_{m}") for m in range(NT)]
        yac = [small.tile([128, 2], F32, tag=f"yac{m}", name=f"yac{m}") for m in range(NT)]
        yac2 = [persist.tile([128, 1], F32, name=f"yac2{m}") for m in range(NT)]
        conv3x3(c1_d, pad1, "c1", y2, yac, sq_acc=yac2)
        st2 = gn_affine_320(make_m2(y2, yac, "gn2", sq_acc=yac2), g2_g_sb, g2_b_sb, "gn2")
        pad2 = [persist.tile([128, 34, 34], BF16, tag=f"pad1_{m}", name=f"pad2_{m}") for m in range(NT)]
        for m in range(NT):
            nc.vector.memset(pad2[m][:], 0.0)
            s, t = st2[m]
            nc.scalar.activation(out=pad2[m][:, 1:33, 1:33],
                                 in_=y2[m][:].rearrange("p (h w) -> p h w", w=32),
                                 func=AF.Relu, bias=t[:, 0:1], scale=s[:, 0:1])

        # ---------------- conv2 + residual (identity matmul) ----------------
        cw2 = [[wstream.tile([128, 9, 128], BF16, tag=f"cw_{k}_{m}",
                             name=f"c2w_{k}_{m}") for m in range(NT)]
               for k in range(NT)]
        for k in range(NT):
            for m in range(NT):
                nc.sync.dma_start(out=cw2[k][m][:], in_=c2_d[k, m])
        for m in range(NT):
            hi = min(128, CH - m * 128)
            for n in range(2):
                r0 = n * 16
                ps = psum.tile([128, 512], F32, tag="mm")
                nc.tensor.matmul(ps[:], eye_sb[:], xbf_sb[m][:, n * 512:(n + 1) * 512],
                                 start=True, stop=False)
                for tap in range(9):
                    dy, dx = tap // 3, tap % 3
                    for k in range(NT):
                        nc.tensor.matmul(
                            ps[:], cw2[k][m][:, tap, :],
                            pad2[k][:, r0 + dy:r0 + dy + 16, dx:dx + 32],
                            start=False, stop=(tap == 8 and k == NT - 1))
                fin = stage.tile([128, 512], F32, tag="fin", bufs=1)
                nc.scalar.activation(out=fin[:], in_=ps[:], func=AF.Copy)
                nc.sync.dma_start(out=y_d[m * 128:m * 128 + hi, n * 512:(n + 1) * 512],
                                  in_=fin[:hi, :])
        es.close()

    nc.compile()
    return nc


_PROGS = {}
_LAST_RESULTS = None
_LAST_EXEC_NS = None


def _get_prog(with_pos):
    if with_pos not in _PROGS:
        _PROGS[with_pos] = build_program(with_pos)
    return _PROGS[with_pos]


def _prep_host(inputs, with_pos):
    """Precompute folded weights; returns the common (weight) part of in_map."""
    f32 = np.float32
    bf16 = ml_dtypes.bfloat16
    w_in = np.asarray(inputs["w_in"], f32)
    wq = np.asarray(inputs["wq"], f32)
    wk = np.asarray(inputs["wk"], f32)
    wv = np.asarray(inputs["wv"], f32)
    wout = np.asarray(inputs["w_attn_out"], f32)
    pos = np.asarray(inputs["pos_emb"], f32)
    scale = HD ** -0.5

    def pad_to(a, shape):
        out = np.zeros(shape, a.dtype)
        out[tuple(slice(0, s) for s in a.shape)] = a
        return out

    def tile_km(mat_t, kt, mt):  # mat_t: [K, M] -> [kt, mt, 128, 128]
        p = pad_to(mat_t, (kt * 128, mt * 128))
        return np.ascontiguousarray(
            p.reshape(kt, 128, mt, 128).transpose(0, 2, 1, 3))

    w_in_tiles = tile_km(w_in.T, NT, NT).astype(bf16)

    # wqktj: per head, wqk_n^T [320->384, 64] tiled into [HN, NT, 128, 64]
    wqktj = np.zeros((HN, NT, 128, CD), f32)
    for n in range(HN):
        wqk_n = scale * (wk[n * HD:(n + 1) * HD, :].T @ wq[n * HD:(n + 1) * HD, :])
        wqktj[n] = pad_to(wqk_n.T, (NT * 128, CD)).reshape(NT, 128, CD)
    wqktj = wqktj.astype(bf16)

    wov = np.concatenate(
        [wout[:, n * HD:(n + 1) * HD] @ wv[n * HD:(n + 1) * HD, :]
         for n in range(HN)], axis=1)          # [320, 512]
    wov_tiles = tile_km(wov.T, 4, NT).astype(bf16)

    def conv_tiles(w):  # [o, i, 3, 3] -> [kt, mt, 128, 9, 128] (tap-interleaved lhsT)
        taps = np.stack([tile_km(np.ascontiguousarray(w[:, :, t // 3, t % 3].T), NT, NT)
                         for t in range(9)], axis=0)  # [9, kt, mt, 128, 128]
        return np.ascontiguousarray(taps.transpose(1, 2, 3, 0, 4)).astype(bf16)

    gsel8 = np.zeros((NT * 128, 8), f32)
    gselT = np.zeros((8, NT * 128), f32)
    for g in range(8):
        gsel8[g * 40:(g + 1) * 40, g] = 1.0 / (40 * NPIX)
        gselT[g, g * 40:(g + 1) * 40] = 1.0
    # ctx group aggregation (8 groups of 8 within 64ch, dup'd); bn_aggr
    # supplies per-channel (mean, E2) so entries are 1/8.
    g2ctx = np.zeros((128, 128), f32)
    for h in range(2):
        for g in range(8):
            sl = slice(h * 64 + g * 8, h * 64 + (g + 1) * 8)
            g2ctx[sl, sl] = 1.0 / 8

    mask32 = np.zeros((128, 32), f32)
    mask32[0:64, 0] = 1.0
    mask32[64:128, 1] = 1.0
    psel = np.zeros((98, 8), f32)
    for n in range(HN):
        psel[32 * (n // 2) + (n % 2), n] = 1.0

    def col(v):
        return pad_to(np.asarray(v, f32).reshape(-1, 1), (384, 1))

    wctx_t = np.ascontiguousarray(np.asarray(inputs["w_ctx"], f32).T)

    common = {
        "w_in_t": w_in_tiles,
        "b_in": col(inputs["b_in"]),
        "gin_g": col(inputs["gn_in_g"]), "gin_b": col(inputs["gn_in_b"]),
        "wctxdup": np.tile(np.concatenate([wctx_t, wctx_t], axis=1), (2, 1)).astype(bf16),
        "gctx_g": np.tile(np.asarray(inputs["gn_ctx_g"], f32), 2).reshape(128, 1),
        "gctx_b": np.tile(np.asarray(inputs["gn_ctx_b"], f32), 2).reshape(128, 1),
        "wqktj": wqktj,
        "wovt": wov_tiles,
        "g1_g": col(inputs["gn1_g"]), "g1_b": col(inputs["gn1_b"]),
        "g2_g": col(inputs["gn2_g"]), "g2_b": col(inputs["gn2_b"]),
        "conv1_t": conv_tiles(np.asarray(inputs["conv1_w"], f32)),
        "conv2_t": conv_tiles(np.asarray(inputs["conv2_w"], f32)),
        "gsel8": np.ascontiguousarray(gsel8.reshape(NT, 128, 8)),
        "gselT": np.ascontiguousarray(gselT.reshape(8, NT, 128).transpose(1, 0, 2)),
        "g2ctx": g2ctx,
        "eye": np.eye(128, dtype=bf16),
        "mask32": mask32.astype(bf16),
        "psel": psel.astype(bf16),
    }
    if with_pos:
        # posT [j, d] duplicated on both partition halves, f32
        posT = np.ascontiguousarray(pos.T)  # [cdim=64, depth=32]
        common["pos_dup"] = np.concatenate([posT, posT], axis=0)
    return common


def kernel(**inputs):
    with_pos = bool(np.any(np.asarray(inputs["pos_emb"])))
    nc = _get_prog(with_pos)
    common = _prep_host(inputs, with_pos)
    x = np.asarray(inputs["x"], np.float32)      # [6, 320, 32, 32]
    ctx = np.asarray(inputs["context"], np.float32)  # [6, 64, 32, 32, 32]
    b = x.shape[0]
    in_maps = []
    for core in range(8):
        s = core if core < b else core - b
        m = dict(common)
        xs = np.zeros((384, NPIX), np.float32)
        xs[:CH] = x[s].reshape(CH, NPIX)
        m["x_bf"] = xs.astype(ml_dtypes.bfloat16)
        m["ctxin"] = np.ascontiguousarray(
            ctx[s].reshape(CD, D * NPIX)).astype(ml_dtypes.bfloat16)
        in_maps.append(m)
    trace = bool(int(os.environ.get("DT_TRACE", "0")))
    kw = {}
    if trace:
        import sys
        import types
        try:
            import antenv.axon_hooks  # noqa: F401
        except ImportError:
            from trn_agent_boot.trn_boot import _ntff_profile_via_ctypes
            m = types.ModuleType("antenv.axon_hooks")
            _h = _ntff_profile_via_ctypes("/opt/axon/libaxon_pjrt.so")
            m.get_axon_ntff_profile_hook = lambda: _h
            sys.modules["antenv.axon_hooks"] = m
        kw = dict(trace=True, tmpdir=os.environ.get("DT_TRACE_DIR") or None)
    res = run_bass_kernel_spmd(nc, in_maps, list(range(8)), **kw)
    global _LAST_RESULTS, _LAST_EXEC_NS
    _LAST_RESULTS = res.results
    _LAST_EXEC_NS = res.exec_time_ns
    if trace:
        print(f"HW exec time: {res.exec_time_ns} ns")
    out = np.stack([res.results[s]["y"] for s in range(b)], axis=0)
    return out.reshape(b, CH, 32, 32).astype(np.float32)


if __name__ == "__main__":
    pass
